# revision 39
# baseline (speedup 1.0000x reference)
"""DualSlidingWindowAttention Trainium2 kernel.

Sharding: 8 cores = 2 batches x 4 head-groups. Core (b, m) owns batch b,
q-heads 8m..8m+7, kv-heads 2m, 2m+1. Host sums the 4 partial o-proj outputs
per batch.

Per-core device program (identical SPMD program, per-core data):
  Phase 1: projections with weights stationary -> transposed outputs
           (qT, kT land score-ready; v is DMA-transposed to [kv, D] via the
           HWDGE xbar, keeping the PE free). All xt tiles get distinct SBUF
           buffers so the input stream prefetches the whole run.
  Phase 2: block-sparse attention. Per (kv-group, 128-query tile) only 5
           128-wide kv chunks matter (3 attn-window from hidden + 2
           ssm-window from ssm). Scores are computed transposed [kv, q] with
           the 4 heads of the group interleaved in the free dim (N=512).
           Softmax: exp(s/8) on ACT (no max subtraction; scores bounded),
           then multiplicative mask*exp(alibi) tile on DVE (GPSIMD takes 1
           in 4 units), softmax sums via a ones-column appended to v (free
           on the PE). Normalization is per-qtile-pair: Z rows round-trip
           through a small DRAM tile for the (t,pr,j)->(pr,c,j) relayout,
           reciprocal on DVE, then a K=2 selector matmul broadcasts 1/Z
           across partitions (no per-unit broadcast DMAs).
  Phase 3: o-proj in qtile-pair chunks (N=256) interleaved into the unit
           loop so the PE stays dense (HAM stays unthrottled) and the tail
           after the last attention unit is short. Output is stored f16;
           the host accumulates partials in f32.

All matmul operands are fp16 (1 cycle/row on the PE, FWL weight loads,
half-sized DMA) except the tiny f32 selector broadcast; accumulation is
always fp32 in PSUM; softmax sums and reciprocals stay fp32.
"""

import sys

sys.path.insert(0, "/opt/trn_rl_repo")

import numpy as np
import concourse.bass as bass
import concourse.bacc as bacc
import concourse.mybir as mybir
import concourse.tile as tile

F32 = mybir.dt.float32
F16 = mybir.dt.float16

HID, H, HK, G, D, T = 2048, 32, 8, 4, 64, 1024
W_ATT, W_SSM = 256, 64
NQT = T // 128  # 8 query tiles
KVG = 2         # kv heads (= head groups) per core
HL = 4          # q heads per kv group

# slot order: [attn_left, ssm_left, attn_full, attn_causal, ssm_causal]
SLOT_SRC = [1, 0, 1, 1, 0]       # 1 = hidden (attn window), 0 = ssm
SLOT_CHOFF = [-2, -1, -1, 0, 0]  # kv chunk offset relative to qtile
SLOT_OFF = [-256, -128, -128, 0, 0]
SLOT_WIN = [W_ATT, W_SSM, W_ATT, W_ATT, W_SSM]


def first_slot(qt):
    return {0: 3, 1: 1}.get(qt, 0)


def build_program(debug_dump=False):
    nc = bacc.Bacc("TRN2", target_bir_lowering=False, debug=False)

    xt_ssm = nc.declare_dram_parameter("xt_ssm", [HID, T], F16, isOutput=False)
    xt_hid = nc.declare_dram_parameter("xt_hid", [HID, T], F16, isOutput=False)
    wq = nc.declare_dram_parameter("wq", [128, 32, 512], F16, isOutput=False)
    wk = nc.declare_dram_parameter("wk", [128, 16, 128], F16, isOutput=False)
    wv = nc.declare_dram_parameter("wv", [128, 16, 128], F16, isOutput=False)
    wsk = nc.declare_dram_parameter("wsk", [128, 16, 128], F16, isOutput=False)
    wsv = nc.declare_dram_parameter("wsv", [128, 16, 128], F16, isOutput=False)
    wo = nc.declare_dram_parameter("wo", [128, 4, 2048], F16, isOutput=False)
    mconc = nc.declare_dram_parameter("mconc", [128, 10, 512], F16, isOutput=False)
    ident = nc.declare_dram_parameter("ident", [128, 128], F16, isOutput=False)
    sel = nc.declare_dram_parameter("sel", [2, 128], F32, isOutput=False)
    out_t = nc.declare_dram_parameter("out_t", [HID, T], F16, isOutput=True)
    if debug_dump:
        zrow_d = nc.declare_dram_parameter("zrow_d", [2, 4, T], F32,
                                           isOutput=True)
        oT_d = nc.declare_dram_parameter("oT_d", [128, 4, T], F32,
                                         isOutput=True)
        oTb_d = nc.declare_dram_parameter("oTb_d", [128, 4, T], F16,
                                          isOutput=True)

    mm = nc.tensor.matmul

    with tile.TileContext(nc) as tc:
        with (
            tc.tile_pool(name="persist", bufs=1) as pers,
        ):
            # persistent sbuf tiles
            qT_sb = pers.tile([128, NQT, HL * 128], F16, tag="qT")
            kT_sb = [pers.tile([128, T], F16, tag=f"kT{s}", name=f"kT{s}")
                     for s in range(2)]
            # v_sb[src][kvh]: [tok-in-chunk, chunk, D+1]; col 64 = ones
            v_sb = [
                [pers.tile([128, NQT, 65], F16, tag=f"v{s}{h}", name=f"v{s}{h}")
                 for h in range(2)]
                for s in range(2)
            ]
            sel_sb = pers.tile([2, 128], F32, tag="sel")
            ident_sb = pers.tile([128, 128], F16, tag="ident")
            oT_sb = pers.tile([128, 4, T], F32, tag="oT")
            oTb_sb = pers.tile([128, 4, T], F16, tag="oTb")
            m_sb = pers.tile([128, 10, 512], F16, tag="mconc")
            wo_sb = pers.tile([128, 4, 2048], F16, tag="wo")
            # Z rows relaid to [pr, c=(kvg,t), tok] for the selector broadcast
            zrow_sb = pers.tile([2, 4, T], F32, tag="zrow")

            # ones columns of v (softmax-sum rows) — set once
            for vsrc in range(2):
                for vh in range(2):
                    nc.vector.memset(v_sb[vsrc][vh][:, :, 64:65], 1.0)

            units = [(kvg, qt) for qt in range(NQT) for kvg in range(KVG)]
            wei_tiles = {}

            # SBUF pools spanning attention (overlap region + back half)
            attn_sbuf = (
                tc.tile_pool(name="weip", bufs=3),
                tc.tile_pool(name="ostgp", bufs=2),
                tc.tile_pool(name="outstgp", bufs=3),
                tc.tile_pool(name="recipp", bufs=2),
            )
            weip, ostgp, outstgp, recipp = (p.__enter__() for p in attn_sbuf)

            def emit_scores(u, sp):
                kvg, qt = units[u]
                fs = first_slot(qt)
                wei_t = weip.tile([128, 5, 512], F16, tag="wei")
                wei_tiles[u] = wei_t
                for s in range(fs, 5):
                    ch = qt + SLOT_CHOFF[s]
                    sp_t = sp.tile([128, 512], F32, tag="sp")
                    mm(sp_t[:, :],
                       lhsT=kT_sb[SLOT_SRC[s]][kvg * 64:(kvg + 1) * 64,
                                               ch * 128:(ch + 1) * 128],
                       rhs=qT_sb[kvg * 64:(kvg + 1) * 64, qt, :],
                       start=True, stop=True)
                    nc.scalar.activation(
                        out=wei_t[:, s, :], in_=sp_t[:, :],
                        func=mybir.ActivationFunctionType.Exp, scale=0.125)
                nc.vector.tensor_mul(
                    wei_t[:, fs:5, :], wei_t[:, fs:5, :],
                    m_sb[:, kvg * 5 + fs:kvg * 5 + 5, :])

            def emit_o(u, op, evac=None):
                kvg, qt = units[u]
                fs = first_slot(qt)
                wei_t = wei_tiles.pop(u)
                op_t = op.tile([128, 512], F32, tag="op")
                for s in range(fs, 5):
                    ch = qt + SLOT_CHOFF[s]
                    mm(op_t[0:65, :],
                       lhsT=v_sb[SLOT_SRC[s]][kvg][:, ch, :],
                       rhs=wei_t[:, s, :],
                       start=(s == fs), stop=(s == 4))
                ostg = ostgp.tile([128, 512], F32, tag="ostg")
                if evac is nc.vector:
                    nc.vector.tensor_copy(ostg[0:65, :], op_t[0:65, :])
                else:
                    nc.scalar.copy(ostg[0:65, :], op_t[0:65, :])
                # Z row (free layout (t, pr, j)) -> zrow[pr, (kvg,t), tok].
                # These 4 small DMAs ride the (otherwise idle) SWDGE queue so
                # they don't serialize the sync HWDGE ring (~0.6us apiece).
                zsrc = ostg[64:65, :].rearrange(
                    "p (t pr j) -> p t pr j", t=2, pr=2)
                for par in range(2):
                    nc.gpsimd.dma_start(
                        out=zrow_sb[par:par + 1, kvg * 2:kvg * 2 + 2,
                                    qt * 128:(qt + 1) * 128],
                        in_=zsrc[:, :, par, :])
                for par in range(2):
                    src_ap = ostg[0:64, :].rearrange(
                        "p (t pr j) -> p t pr j", t=2, pr=2)[:, :, par, :]
                    dst_ap = oT_sb[par * 64:(par + 1) * 64,
                                   kvg * 2:kvg * 2 + 2,
                                   qt * 128:(qt + 1) * 128]
                    nc.gpsimd.dma_start(out=dst_ap, in_=src_ap)

            # ---------------- Phase 1 + overlapped attention ----------------
            with (
                tc.tile_pool(name="wqp", bufs=1) as wqp,
                tc.tile_pool(name="xtp", bufs=40) as xtp,
                tc.tile_pool(name="stgp", bufs=2) as stgp,
                tc.tile_pool(name="qp", bufs=4, space="PSUM") as qp,
            ):
                w4_names = ("wsk", "wsv", "wk", "wv")
                w4_t = {"wsk": wsk, "wsv": wsv, "wk": wk, "wv": wv}
                w4_sb = {}
                for name in w4_names:
                    w4_sb[name] = wqp.tile([128, 16, 128], F16, tag=name, name=name)
                wq_sb = [wqp.tile([128, 32, 128], F16, tag=f"wq{c}", name=f"wq{c}")
                         for c in range(4)]

                # DMA emission order = consumption order so the single HWDGE
                # queue streams without head-of-line blocking.
                nc.sync.dma_start(out=w4_sb["wsk"], in_=wsk[:, :, :])
                nc.sync.dma_start(out=w4_sb["wsv"], in_=wsv[:, :, :])
                xts = {}

                def load_xt(half, src):
                    xt_t = xt_hid if src else xt_ssm
                    for kc in range(16):
                        xtile = xtp.tile([128, 512], F16, tag="xt",
                                         name=f"xt{half}_{src}_{kc}")
                        nc.sync.dma_start(
                            out=xtile,
                            in_=xt_t[kc * 128:(kc + 1) * 128,
                                     half * 512:(half + 1) * 512])
                        xts[(half, src, kc)] = xtile

                def load_wq(c):
                    nc.sync.dma_start(
                        out=wq_sb[c][:, :, :],
                        in_=wq[:, :, c * 128:(c + 1) * 128])

                load_wq(0)
                load_xt(0, 0)
                nc.sync.dma_start(out=w4_sb["wk"], in_=wk[:, :, :])
                nc.sync.dma_start(out=w4_sb["wv"], in_=wv[:, :, :])
                load_wq(1)
                load_xt(0, 1)
                load_wq(2)
                load_wq(3)
                load_xt(1, 0)
                load_xt(1, 1)
                nc.sync.dma_start(out=ident_sb, in_=ident[:, :])
                nc.sync.dma_start(out=m_sb, in_=mconc[:, :, :])
                nc.sync.dma_start(out=sel_sb, in_=sel[:, :])
                for c4 in range(4):
                    nc.sync.dma_start(out=wo_sb[:, c4, :], in_=wo[:, c4, :])

                def emit_q(qps, half, c, src):
                    for kc in range(16):
                        mm(qps[c][:, :],
                           lhsT=wq_sb[c][:, src * 16 + kc, :],
                           rhs=xts[(half, src, kc)][:, :],
                           start=(src == 0 and kc == 0),
                           stop=(src == 1 and kc == 15))

                def evac_q(qps, half):
                    # host permutes Wq cols so col-tile c = [head c (kvg0),
                    # head 4+c (kvg1)] -> partition p maps to p directly.
                    for c in range(4):
                        nc.vector.tensor_copy(
                            qT_sb[:, half * 4:(half + 1) * 4,
                                  c * 128:(c + 1) * 128],
                            qps[c][:, :].rearrange("p (qt j) -> p qt j", j=128))

                qps_h = [
                    [qp.tile([128, 512], F32, tag="qps", name=f"qps{hf}_{i}")
                     for i in range(4)]
                    for hf in range(2)
                ]

                with (
                    tc.tile_pool(name="kvp", bufs=2, space="PSUM") as kvp,
                    tc.tile_pool(name="tp", bufs=2, space="PSUM") as tp,
                ):
                    def emit_kv(half, src):
                        kps = kvp.tile([128, 512], F32, tag="kvps")
                        vps = kvp.tile([128, 512], F32, tag="kvps")
                        wk_t = w4_sb["wk" if src else "wsk"]
                        wv_t = w4_sb["wv" if src else "wsv"]
                        for kc in range(16):
                            xtile = xts[(half, src, kc)]
                            mm(kps[:, :], lhsT=wk_t[:, kc, :], rhs=xtile[:, :],
                               start=(kc == 0), stop=(kc == 15))
                            mm(vps[:, :], lhsT=wv_t[:, kc, :], rhs=xtile[:, :],
                               start=(kc == 0), stop=(kc == 15))
                        nc.vector.tensor_copy(
                            kT_sb[src][:, half * 512:(half + 1) * 512],
                            kps[:, :])
                        vstg = stgp.tile([128, 512], F16, tag="vstg")
                        nc.vector.tensor_copy(vstg[:, :], vps[:, :])
                        # v -> [tok, D] via PE transpose
                        for h in range(2):
                            for j4 in range(4):
                                tp_t = tp.tile([128, 64], F16, tag="tp")
                                nc.tensor.transpose(
                                    tp_t[:, :],
                                    vstg[h * 64:(h + 1) * 64,
                                         j4 * 128:(j4 + 1) * 128],
                                    ident_sb[h * 64:(h + 1) * 64,
                                             h * 64:(h + 1) * 64])
                                nc.scalar.copy(
                                    v_sb[src][h][:, half * 4 + j4, 0:64],
                                    tp_t[:, :])

                    # half 0 fully (kv + q, c0 interleaved so the PE has q
                    # work as soon as wq chunk 0 lands); half 1 kv only.
                    emit_kv(0, 0)
                    emit_q(qps_h[0], 0, 0, 0)
                    emit_kv(0, 1)
                    emit_q(qps_h[0], 0, 0, 1)
                    for c in range(1, 4):
                        emit_q(qps_h[0], 0, c, 0)
                        emit_q(qps_h[0], 0, c, 1)
                    evac_q(qps_h[0], 0)
                    emit_kv(1, 0)
                    emit_kv(1, 1)

                def emit_recip(p):
                    # Z is a positive normal (1 .. ~2e4): approx-fast is safe
                    rc32 = recipp.tile([2, 4, 256], F32, tag="rc32")
                    nc.vector.reciprocal_approx_fast(
                        out=rc32[:, :, :],
                        in_=zrow_sb[0:2, :, p * 256:(p + 1) * 256])
                    return rc32

                # Overlap region: attention units 0..7 (qtiles 0-3, all from
                # half 0) woven with the half-1 q projection, so the PE stays
                # dense through the phase transition (HAM stays warm).
                rc16s = {}
                with (
                    tc.tile_pool(name="spA", bufs=2, space="PSUM") as spA,
                    tc.tile_pool(name="opA", bufs=2, space="PSUM") as opA,
                ):
                    weave = [(0, 0), (0, 1), (1, 0), (1, 1),
                             (2, 0), (2, 1), (3, 0), (3, 1)]
                    for u in range(8):
                        emit_scores(u, spA)
                        c, src = weave[u]
                        emit_q(qps_h[1], 1, c, src)
                        if u > 1:
                            emit_o(u - 2, opA)
                        if u == 7:
                            rc16s[0] = emit_recip(0)
                    emit_o(6, opA)
                    emit_o(7, opA)
                    evac_q(qps_h[1], 1)
                    rc16s[1] = emit_recip(1)

            # ---------------- back half: units 8..15 + all norms/o-proj ----
            with (
                tc.tile_pool(name="spB", bufs=3, space="PSUM") as spB,
                tc.tile_pool(name="opB", bufs=2, space="PSUM") as opB,
                tc.tile_pool(name="rbcp", bufs=1, space="PSUM") as rbcp,
                tc.tile_pool(name="p3", bufs=2, space="PSUM") as p3p,
            ):
                def emit_bcast(p):
                    rc32 = rc16s.pop(p)
                    for c in range(4):
                        rb = rbcp.tile([128, 256], F32, tag="rbc")
                        # selector matmul: out[p, f] = rc[p // 64, c, f]
                        mm(rb[:, :], lhsT=sel_sb[0:2, :], rhs=rc32[0:2, c, :],
                           start=True, stop=True)
                        nc.vector.tensor_mul(
                            oTb_sb[:, c, p * 256:(p + 1) * 256],
                            oT_sb[:, c, p * 256:(p + 1) * 256],
                            rb[:, :])

                def emit_oproj_pair(p, ns=range(16)):
                    for n in ns:
                        ps = p3p.tile([128, 256], F32, tag="p3")
                        for c in range(4):
                            mm(ps[:, :],
                               lhsT=wo_sb[:, c, n * 128:(n + 1) * 128],
                               rhs=oTb_sb[:, c, p * 256:(p + 1) * 256],
                               start=(c == 0), stop=(c == 3))
                        og = outstgp.tile([128, 256], F16, tag="outstg")
                        if n % 2 == 0:
                            nc.scalar.copy(og[:, :], ps[:, :])
                        else:
                            nc.vector.tensor_copy(og[:, :], ps[:, :])
                        dma_eng = nc.sync if n % 2 == 0 else nc.scalar
                        dma_eng.dma_start(
                            out=out_t[n * 128:(n + 1) * 128,
                                      p * 256:(p + 1) * 256],
                            in_=og[:, :])

                for u in range(8, 16):
                    emit_scores(u, spB)
                    if u >= 10:
                        emit_o(u - 2, opB, evac=nc.vector)
                    if u == 8:
                        emit_bcast(0)
                        emit_oproj_pair(0)
                    if u == 9:
                        emit_bcast(1)
                        emit_oproj_pair(1)
                    if u == 13:
                        rc16s[2] = emit_recip(2)
                        emit_bcast(2)
                        emit_oproj_pair(2, range(0, 8))
                # spread oproj2's back 8 n-tiles through the o14/o15 stretch
                # so the PE stays dense (HAM warm) right into the tail.
                emit_o(14, opB, evac=nc.vector)
                emit_oproj_pair(2, range(8, 12))
                emit_o(15, opB, evac=nc.vector)
                emit_oproj_pair(2, range(12, 16))
                rc16s[3] = emit_recip(3)
                emit_bcast(3)
                emit_oproj_pair(3)
                if debug_dump:
                    nc.sync.dma_start(out=zrow_d[:, :, :], in_=zrow_sb[:, :, :])
                    nc.sync.dma_start(out=oT_d[:, :, :], in_=oT_sb[:, :, :])
                    nc.sync.dma_start(out=oTb_d[:, :, :], in_=oTb_sb[:, :, :])

            for p_cm in reversed(attn_sbuf):
                p_cm.__exit__(None, None, None)

    nc.finalize()
    return nc


def make_mconc(m):
    """Mask*exp(alibi) tile for core head-group m: [128, 10, 512] f16."""
    p = np.arange(128)[:, None]
    j = np.arange(128)[None, :]
    out = np.zeros((128, 10, 512), np.float16)
    for kvg in range(KVG):
        for s in range(5):
            rel = SLOT_OFF[s] + p - j  # [128, 128] kv - q
            mask = (-rel >= 0) & (-rel < SLOT_WIN[s])
            for hl in range(HL):
                hg = 8 * m + kvg * 4 + hl
                slope = 2.0 ** (-8.0 * hg / H)
                vals = np.where(mask, np.exp(slope * rel.astype(np.float64)), 0.0)
                out[:, kvg * 5 + s, hl * 128:(hl + 1) * 128] = vals.astype(np.float16)
    return out


def make_inputs(core, hidden_states, ssm_states, Wq, Wk, Wv, Wsk, Wsv, Wo):
    b, m = core // 4, core % 4
    f16 = lambda x: np.ascontiguousarray(np.asarray(x, dtype=np.float16))

    def wshard(W, cols, nchunk):
        # [K, cols] -> [128, K//128, cols]
        Ws = np.asarray(W)[:, cols]
        return f16(Ws.reshape(nchunk, 128, Ws.shape[1]).transpose(1, 0, 2))

    # col-tile c = [head c (kvg0) cols, head 4+c (kvg1) cols]
    qperm = np.concatenate(
        [np.arange(64) + 64 * h for c in range(4) for h in (c, 4 + c)])
    qcols = 512 * m + qperm
    kvcols = slice(128 * m, 128 * (m + 1))
    wo_sh = np.asarray(Wo)[512 * m:512 * (m + 1), :]
    sel = np.zeros((2, 128), np.float32)
    sel[0, 0:64] = 1.0
    sel[1, 64:128] = 1.0
    return {
        "xt_ssm": f16(np.asarray(ssm_states)[b].T),
        "xt_hid": f16(np.asarray(hidden_states)[b].T),
        "wq": wshard(Wq, qcols, 32),
        "wk": wshard(Wk, kvcols, 16),
        "wv": wshard(Wv, kvcols, 16),
        "wsk": wshard(Wsk, kvcols, 16),
        "wsv": wshard(Wsv, kvcols, 16),
        "wo": f16(wo_sh.reshape(4, 128, 2048).transpose(1, 0, 2)),
        "mconc": make_mconc(m),
        "ident": np.eye(128, dtype=np.float16),
        "sel": sel,
    }


def gather(results):
    out = np.zeros((2, T, HID), np.float32)
    for core in range(8):
        b = core // 4
        out[b] += results[core]["out_t"].T.astype(np.float32)
    return out


# ----------------------------------------------------------------------------
# Harness entry point
# ----------------------------------------------------------------------------
_NC_CACHE = []


def _get_program():
    if not _NC_CACHE:
        _NC_CACHE.append(build_program())
    return _NC_CACHE[0]


def _run(inp, trace=False, tmpdir=None):
    from concourse.bass_utils import run_bass_kernel_spmd

    nc = _get_program()
    in_maps = [make_inputs(core, **{k: np.asarray(inp[k]) for k in (
        "hidden_states", "ssm_states", "Wq", "Wk", "Wv", "Wsk", "Wsv", "Wo")})
        for core in range(8)]
    res = run_bass_kernel_spmd(nc, in_maps, list(range(8)), trace=trace,
                               tmpdir=tmpdir)
    return gather(res.results), res.exec_time_ns


def kernel(hidden_states, ssm_states, Wq, Wk, Wv, Wsk, Wsv, Wo):
    out, _ = _run(dict(
        hidden_states=hidden_states, ssm_states=ssm_states, Wq=Wq, Wk=Wk,
        Wv=Wv, Wsk=Wsk, Wsv=Wsv, Wo=Wo))
    return out


# revision 44
# speedup vs baseline: 1.0412x; 1.0412x over previous
"""DualSlidingWindowAttention Trainium2 kernel.

Sharding: 8 cores = 2 batches x 4 head-groups. Core (b, m) owns batch b,
q-heads 8m..8m+7, kv-heads 2m, 2m+1. Host sums the 4 partial o-proj outputs
per batch.

Per-core device program (identical SPMD program, per-core data):
  Phase 1: projections with weights stationary -> transposed outputs
           (qT, kT land score-ready; v is DMA-transposed to [kv, D] via the
           HWDGE xbar, keeping the PE free). All xt tiles get distinct SBUF
           buffers so the input stream prefetches the whole run.
  Phase 2: block-sparse attention. Per (kv-group, 128-query tile) only 5
           128-wide kv chunks matter (3 attn-window from hidden + 2
           ssm-window from ssm). Scores are computed transposed [kv, q] with
           the 4 heads of the group interleaved in the free dim (N=512).
           Softmax: exp(s/8) on ACT (no max subtraction; scores bounded),
           then multiplicative mask*exp(alibi) tile on DVE (GPSIMD takes 1
           in 4 units), softmax sums via a ones-column appended to v (free
           on the PE). Normalization is per-qtile-pair: Z rows round-trip
           through a small DRAM tile for the (t,pr,j)->(pr,c,j) relayout,
           reciprocal on DVE, then a K=2 selector matmul broadcasts 1/Z
           across partitions (no per-unit broadcast DMAs).
  Phase 3: o-proj in qtile-pair chunks (N=256) interleaved into the unit
           loop so the PE stays dense (HAM stays unthrottled) and the tail
           after the last attention unit is short. Output is stored f16;
           the host accumulates partials in f32.

All matmul operands are fp16 (1 cycle/row on the PE, FWL weight loads,
half-sized DMA) except the tiny f32 selector broadcast; accumulation is
always fp32 in PSUM; softmax sums and reciprocals stay fp32.
"""

import sys

sys.path.insert(0, "/opt/trn_rl_repo")

import numpy as np
import concourse.bass as bass
import concourse.bacc as bacc
import concourse.mybir as mybir
import concourse.tile as tile

F32 = mybir.dt.float32
F16 = mybir.dt.float16

HID, H, HK, G, D, T = 2048, 32, 8, 4, 64, 1024
W_ATT, W_SSM = 256, 64
NQT = T // 128  # 8 query tiles
KVG = 2         # kv heads (= head groups) per core
HL = 4          # q heads per kv group

# slot order: [attn_left, ssm_left, attn_full, attn_causal, ssm_causal]
SLOT_SRC = [1, 0, 1, 1, 0]       # 1 = hidden (attn window), 0 = ssm
SLOT_CHOFF = [-2, -1, -1, 0, 0]  # kv chunk offset relative to qtile
SLOT_OFF = [-256, -128, -128, 0, 0]
SLOT_WIN = [W_ATT, W_SSM, W_ATT, W_ATT, W_SSM]


def first_slot(qt):
    return {0: 3, 1: 1}.get(qt, 0)


def build_program(debug_dump=False):
    nc = bacc.Bacc("TRN2", target_bir_lowering=False, debug=False)

    xt_ssm = nc.declare_dram_parameter("xt_ssm", [HID, T], F16, isOutput=False)
    xt_hid = nc.declare_dram_parameter("xt_hid", [HID, T], F16, isOutput=False)
    wq = nc.declare_dram_parameter("wq", [4, 128, 32, 128], F16, isOutput=False)
    wk = nc.declare_dram_parameter("wk", [128, 16, 128], F16, isOutput=False)
    wv = nc.declare_dram_parameter("wv", [128, 16, 128], F16, isOutput=False)
    wsk = nc.declare_dram_parameter("wsk", [128, 16, 128], F16, isOutput=False)
    wsv = nc.declare_dram_parameter("wsv", [128, 16, 128], F16, isOutput=False)
    wo = nc.declare_dram_parameter("wo", [128, 4, 2048], F16, isOutput=False)
    mconc = nc.declare_dram_parameter("mconc", [128, 10, 512], F16, isOutput=False)
    ident = nc.declare_dram_parameter("ident", [128, 128], F16, isOutput=False)
    sel = nc.declare_dram_parameter("sel", [2, 128], F32, isOutput=False)
    out_t = nc.declare_dram_parameter("out_t", [HID, T], F16, isOutput=True)
    if debug_dump:
        zrow_d = nc.declare_dram_parameter("zrow_d", [2, 4, T], F32,
                                           isOutput=True)
        oT_d = nc.declare_dram_parameter("oT_d", [128, 4, T], F32,
                                         isOutput=True)
        oTb_d = nc.declare_dram_parameter("oTb_d", [128, 4, T], F16,
                                          isOutput=True)

    mm = nc.tensor.matmul

    with tile.TileContext(nc) as tc:
        with (
            tc.tile_pool(name="persist", bufs=1) as pers,
        ):
            # persistent sbuf tiles
            qT_sb = pers.tile([128, NQT, HL * 128], F16, tag="qT")
            kT_sb = [pers.tile([128, T], F16, tag=f"kT{s}", name=f"kT{s}")
                     for s in range(2)]
            # v_sb[src][kvh]: [tok-in-chunk, chunk, D+1]; col 64 = ones
            v_sb = [
                [pers.tile([128, NQT, 65], F16, tag=f"v{s}{h}", name=f"v{s}{h}")
                 for h in range(2)]
                for s in range(2)
            ]
            sel_sb = pers.tile([2, 128], F32, tag="sel")
            ident_sb = pers.tile([128, 128], F16, tag="ident")
            oT_sb = pers.tile([128, 4, T], F32, tag="oT")
            oTb_sb = pers.tile([128, 4, T], F16, tag="oTb")
            m_sb = pers.tile([128, 10, 512], F16, tag="mconc")
            wo_sb = pers.tile([128, 4, 2048], F16, tag="wo")
            # Z rows relaid to [pr, c=(kvg,t), tok] for the selector broadcast
            zrow_sb = pers.tile([2, 4, T], F32, tag="zrow")

            # ones columns of v (softmax-sum rows) — set once
            for vsrc in range(2):
                for vh in range(2):
                    nc.vector.memset(v_sb[vsrc][vh][:, :, 64:65], 1.0)

            units = [(kvg, qt) for qt in range(NQT) for kvg in range(KVG)]
            wei_tiles = {}

            # SBUF pools spanning attention (overlap region + back half)
            attn_sbuf = (
                tc.tile_pool(name="weip", bufs=3),
                tc.tile_pool(name="ostgp", bufs=2),
                tc.tile_pool(name="outstgp", bufs=3),
                tc.tile_pool(name="recipp", bufs=2),
            )
            weip, ostgp, outstgp, recipp = (p.__enter__() for p in attn_sbuf)

            def emit_scores(u, sp):
                kvg, qt = units[u]
                fs = first_slot(qt)
                wei_t = weip.tile([128, 5, 512], F16, tag="wei")
                wei_tiles[u] = wei_t
                for s in range(fs, 5):
                    ch = qt + SLOT_CHOFF[s]
                    sp_t = sp.tile([128, 512], F32, tag="sp")
                    mm(sp_t[:, :],
                       lhsT=kT_sb[SLOT_SRC[s]][kvg * 64:(kvg + 1) * 64,
                                               ch * 128:(ch + 1) * 128],
                       rhs=qT_sb[kvg * 64:(kvg + 1) * 64, qt, :],
                       start=True, stop=True)
                    nc.scalar.activation(
                        out=wei_t[:, s, :], in_=sp_t[:, :],
                        func=mybir.ActivationFunctionType.Exp, scale=0.125)
                nc.vector.tensor_mul(
                    wei_t[:, fs:5, :], wei_t[:, fs:5, :],
                    m_sb[:, kvg * 5 + fs:kvg * 5 + 5, :])

            def emit_o(u, op, evac=None):
                kvg, qt = units[u]
                fs = first_slot(qt)
                wei_t = wei_tiles.pop(u)
                op_t = op.tile([128, 512], F32, tag="op")
                for s in range(fs, 5):
                    ch = qt + SLOT_CHOFF[s]
                    mm(op_t[0:65, :],
                       lhsT=v_sb[SLOT_SRC[s]][kvg][:, ch, :],
                       rhs=wei_t[:, s, :],
                       start=(s == fs), stop=(s == 4))
                ostg = ostgp.tile([128, 512], F32, tag="ostg")
                if evac is nc.vector:
                    nc.vector.tensor_copy(ostg[0:65, :], op_t[0:65, :])
                else:
                    nc.scalar.copy(ostg[0:65, :], op_t[0:65, :])
                # Z row (free layout (t, pr, j)) -> zrow[pr, (kvg,t), tok].
                # These 4 small DMAs ride the (otherwise idle) SWDGE queue so
                # they don't serialize the sync HWDGE ring (~0.6us apiece).
                zsrc = ostg[64:65, :].rearrange(
                    "p (t pr j) -> p t pr j", t=2, pr=2)
                for par in range(2):
                    nc.gpsimd.dma_start(
                        out=zrow_sb[par:par + 1, kvg * 2:kvg * 2 + 2,
                                    qt * 128:(qt + 1) * 128],
                        in_=zsrc[:, :, par, :])
                for par in range(2):
                    src_ap = ostg[0:64, :].rearrange(
                        "p (t pr j) -> p t pr j", t=2, pr=2)[:, :, par, :]
                    dst_ap = oT_sb[par * 64:(par + 1) * 64,
                                   kvg * 2:kvg * 2 + 2,
                                   qt * 128:(qt + 1) * 128]
                    nc.gpsimd.dma_start(out=dst_ap, in_=src_ap)

            # ---------------- Phase 1 + overlapped attention ----------------
            with (
                tc.tile_pool(name="wqp", bufs=1) as wqp,
                tc.tile_pool(name="xtp", bufs=40) as xtp,
                tc.tile_pool(name="stgp", bufs=2) as stgp,
                tc.tile_pool(name="qp", bufs=4, space="PSUM") as qp,
            ):
                w4_names = ("wsk", "wsv", "wk", "wv")
                w4_t = {"wsk": wsk, "wsv": wsv, "wk": wk, "wv": wv}
                w4_sb = {}
                for name in w4_names:
                    w4_sb[name] = wqp.tile([128, 16, 128], F16, tag=name, name=name)
                wq_sb = [wqp.tile([128, 32, 128], F16, tag=f"wq{c}", name=f"wq{c}")
                         for c in range(4)]

                # DMA emission order = consumption order so the single HWDGE
                # queue streams without head-of-line blocking.
                nc.sync.dma_start(out=w4_sb["wsk"], in_=wsk[:, :, :])
                nc.sync.dma_start(out=w4_sb["wsv"], in_=wsv[:, :, :])
                xts = {}

                def load_xt(half, src):
                    xt_t = xt_hid if src else xt_ssm
                    for kc in range(16):
                        xtile = xtp.tile([128, 512], F16, tag="xt",
                                         name=f"xt{half}_{src}_{kc}")
                        nc.sync.dma_start(
                            out=xtile,
                            in_=xt_t[kc * 128:(kc + 1) * 128,
                                     half * 512:(half + 1) * 512])
                        xts[(half, src, kc)] = xtile

                def load_wq(c):
                    # c-major host layout: each col-tile is one contiguous
                    # 1MB slab (512B+ per descriptor line => full DMA rate)
                    nc.sync.dma_start(
                        out=wq_sb[c][:, :, :],
                        in_=wq[c, :, :, :])

                load_wq(0)
                load_xt(0, 0)
                nc.sync.dma_start(out=w4_sb["wk"], in_=wk[:, :, :])
                nc.sync.dma_start(out=w4_sb["wv"], in_=wv[:, :, :])
                load_wq(1)
                load_xt(0, 1)
                load_wq(2)
                load_wq(3)
                load_xt(1, 0)
                load_xt(1, 1)
                nc.sync.dma_start(out=ident_sb, in_=ident[:, :])
                nc.sync.dma_start(out=m_sb, in_=mconc[:, :, :])
                nc.sync.dma_start(out=sel_sb, in_=sel[:, :])
                for c4 in range(4):
                    nc.sync.dma_start(out=wo_sb[:, c4, :], in_=wo[:, c4, :])

                def emit_q(qps, half, c, src):
                    for kc in range(16):
                        mm(qps[c][:, :],
                           lhsT=wq_sb[c][:, src * 16 + kc, :],
                           rhs=xts[(half, src, kc)][:, :],
                           start=(src == 0 and kc == 0),
                           stop=(src == 1 and kc == 15))

                def evac_q(qps, half):
                    # host permutes Wq cols so col-tile c = [head c (kvg0),
                    # head 4+c (kvg1)] -> partition p maps to p directly.
                    for c in range(4):
                        nc.vector.tensor_copy(
                            qT_sb[:, half * 4:(half + 1) * 4,
                                  c * 128:(c + 1) * 128],
                            qps[c][:, :].rearrange("p (qt j) -> p qt j", j=128))

                qps_h = [
                    [qp.tile([128, 512], F32, tag="qps", name=f"qps{hf}_{i}")
                     for i in range(4)]
                    for hf in range(2)
                ]

                with (
                    tc.tile_pool(name="kvp", bufs=2, space="PSUM") as kvp,
                    tc.tile_pool(name="tp", bufs=2, space="PSUM") as tp,
                ):
                    def emit_kv(half, src):
                        kps = kvp.tile([128, 512], F32, tag="kvps")
                        vps = kvp.tile([128, 512], F32, tag="kvps")
                        wk_t = w4_sb["wk" if src else "wsk"]
                        wv_t = w4_sb["wv" if src else "wsv"]
                        for kc in range(16):
                            xtile = xts[(half, src, kc)]
                            mm(kps[:, :], lhsT=wk_t[:, kc, :], rhs=xtile[:, :],
                               start=(kc == 0), stop=(kc == 15))
                            mm(vps[:, :], lhsT=wv_t[:, kc, :], rhs=xtile[:, :],
                               start=(kc == 0), stop=(kc == 15))
                        nc.vector.tensor_copy(
                            kT_sb[src][:, half * 512:(half + 1) * 512],
                            kps[:, :])
                        vstg = stgp.tile([128, 512], F16, tag="vstg")
                        nc.vector.tensor_copy(vstg[:, :], vps[:, :])
                        # v -> [tok, D] via PE transpose
                        for h in range(2):
                            for j4 in range(4):
                                tp_t = tp.tile([128, 64], F16, tag="tp")
                                nc.tensor.transpose(
                                    tp_t[:, :],
                                    vstg[h * 64:(h + 1) * 64,
                                         j4 * 128:(j4 + 1) * 128],
                                    ident_sb[h * 64:(h + 1) * 64,
                                             h * 64:(h + 1) * 64])
                                nc.scalar.copy(
                                    v_sb[src][h][:, half * 4 + j4, 0:64],
                                    tp_t[:, :])

                    # half 0 fully (kv + q, c0 interleaved so the PE has q
                    # work as soon as wq chunk 0 lands); half 1 kv only.
                    emit_kv(0, 0)
                    emit_q(qps_h[0], 0, 0, 0)
                    emit_kv(0, 1)
                    emit_q(qps_h[0], 0, 0, 1)
                    for c in range(1, 4):
                        emit_q(qps_h[0], 0, c, 0)
                        emit_q(qps_h[0], 0, c, 1)
                    evac_q(qps_h[0], 0)
                    emit_kv(1, 0)
                    emit_kv(1, 1)

                def emit_recip(p):
                    # Z is a positive normal (1 .. ~2e4): approx-fast is safe
                    rc32 = recipp.tile([2, 4, 256], F32, tag="rc32")
                    nc.vector.reciprocal_approx_fast(
                        out=rc32[:, :, :],
                        in_=zrow_sb[0:2, :, p * 256:(p + 1) * 256])
                    return rc32

                # Overlap region: attention units 0..7 (qtiles 0-3, all from
                # half 0) woven with the half-1 q projection, so the PE stays
                # dense through the phase transition (HAM stays warm).
                rc16s = {}
                with (
                    tc.tile_pool(name="spA", bufs=2, space="PSUM") as spA,
                    tc.tile_pool(name="opA", bufs=2, space="PSUM") as opA,
                ):
                    weave = [(0, 0), (0, 1), (1, 0), (1, 1),
                             (2, 0), (2, 1), (3, 0), (3, 1)]
                    for u in range(8):
                        emit_scores(u, spA)
                        c, src = weave[u]
                        emit_q(qps_h[1], 1, c, src)
                        if u > 1:
                            emit_o(u - 2, opA)
                        if u == 7:
                            rc16s[0] = emit_recip(0)
                    emit_o(6, opA)
                    emit_o(7, opA)
                    evac_q(qps_h[1], 1)
                    rc16s[1] = emit_recip(1)

            # ---------------- back half: units 8..15 + all norms/o-proj ----
            with (
                tc.tile_pool(name="spB", bufs=3, space="PSUM") as spB,
                tc.tile_pool(name="opB", bufs=2, space="PSUM") as opB,
                tc.tile_pool(name="rbcp", bufs=1, space="PSUM") as rbcp,
                tc.tile_pool(name="p3", bufs=2, space="PSUM") as p3p,
            ):
                def emit_bcast(p):
                    rc32 = rc16s.pop(p)
                    for c in range(4):
                        rb = rbcp.tile([128, 256], F32, tag="rbc")
                        # selector matmul: out[p, f] = rc[p // 64, c, f]
                        mm(rb[:, :], lhsT=sel_sb[0:2, :], rhs=rc32[0:2, c, :],
                           start=True, stop=True)
                        nc.vector.tensor_mul(
                            oTb_sb[:, c, p * 256:(p + 1) * 256],
                            oT_sb[:, c, p * 256:(p + 1) * 256],
                            rb[:, :])

                def emit_oproj_pair(p, ns=range(16)):
                    for n in ns:
                        ps = p3p.tile([128, 256], F32, tag="p3")
                        for c in range(4):
                            mm(ps[:, :],
                               lhsT=wo_sb[:, c, n * 128:(n + 1) * 128],
                               rhs=oTb_sb[:, c, p * 256:(p + 1) * 256],
                               start=(c == 0), stop=(c == 3))
                        og = outstgp.tile([128, 256], F16, tag="outstg")
                        if n % 2 == 0:
                            nc.scalar.copy(og[:, :], ps[:, :])
                        else:
                            nc.vector.tensor_copy(og[:, :], ps[:, :])
                        nc.sync.dma_start(
                            out=out_t[n * 128:(n + 1) * 128,
                                      p * 256:(p + 1) * 256],
                            in_=og[:, :])

                for u in range(8, 16):
                    emit_scores(u, spB)
                    if u >= 10:
                        emit_o(u - 2, opB, evac=nc.vector)
                    if u == 8:
                        emit_bcast(0)
                        emit_oproj_pair(0)
                    if u == 9:
                        emit_bcast(1)
                        emit_oproj_pair(1)
                    if u == 13:
                        rc16s[2] = emit_recip(2)
                        emit_bcast(2)
                        emit_oproj_pair(2, range(0, 8))
                # spread oproj2's back 8 n-tiles through the o14/o15 stretch
                # so the PE stays dense (HAM warm) right into the tail.
                emit_o(14, opB, evac=nc.vector)
                emit_oproj_pair(2, range(8, 12))
                emit_o(15, opB, evac=nc.vector)
                emit_oproj_pair(2, range(12, 16))
                rc16s[3] = emit_recip(3)
                emit_bcast(3)
                emit_oproj_pair(3)
                if debug_dump:
                    nc.sync.dma_start(out=zrow_d[:, :, :], in_=zrow_sb[:, :, :])
                    nc.sync.dma_start(out=oT_d[:, :, :], in_=oT_sb[:, :, :])
                    nc.sync.dma_start(out=oTb_d[:, :, :], in_=oTb_sb[:, :, :])

            for p_cm in reversed(attn_sbuf):
                p_cm.__exit__(None, None, None)

    nc.finalize()
    return nc


def make_mconc(m):
    """Mask*exp(alibi) tile for core head-group m: [128, 10, 512] f16."""
    p = np.arange(128)[:, None]
    j = np.arange(128)[None, :]
    out = np.zeros((128, 10, 512), np.float16)
    for kvg in range(KVG):
        for s in range(5):
            rel = SLOT_OFF[s] + p - j  # [128, 128] kv - q
            mask = (-rel >= 0) & (-rel < SLOT_WIN[s])
            for hl in range(HL):
                hg = 8 * m + kvg * 4 + hl
                slope = 2.0 ** (-8.0 * hg / H)
                vals = np.where(mask, np.exp(slope * rel.astype(np.float64)), 0.0)
                out[:, kvg * 5 + s, hl * 128:(hl + 1) * 128] = vals.astype(np.float16)
    return out


def make_inputs(core, hidden_states, ssm_states, Wq, Wk, Wv, Wsk, Wsv, Wo):
    b, m = core // 4, core % 4
    f16 = lambda x: np.ascontiguousarray(np.asarray(x, dtype=np.float16))

    def wshard(W, cols, nchunk):
        # [K, cols] -> [128, K//128, cols]
        Ws = np.asarray(W)[:, cols]
        return f16(Ws.reshape(nchunk, 128, Ws.shape[1]).transpose(1, 0, 2))

    # col-tile c = [head c (kvg0) cols, head 4+c (kvg1) cols]
    qperm = np.concatenate(
        [np.arange(64) + 64 * h for c in range(4) for h in (c, 4 + c)])
    qcols = 512 * m + qperm
    wq_sh = wshard(Wq, qcols, 32)  # [128, 32, 512]
    wq_cmaj = np.ascontiguousarray(
        wq_sh.reshape(128, 32, 4, 128).transpose(2, 0, 1, 3))
    kvcols = slice(128 * m, 128 * (m + 1))
    wo_sh = np.asarray(Wo)[512 * m:512 * (m + 1), :]
    sel = np.zeros((2, 128), np.float32)
    sel[0, 0:64] = 1.0
    sel[1, 64:128] = 1.0
    return {
        "xt_ssm": f16(np.asarray(ssm_states)[b].T),
        "xt_hid": f16(np.asarray(hidden_states)[b].T),
        "wq": wq_cmaj,
        "wk": wshard(Wk, kvcols, 16),
        "wv": wshard(Wv, kvcols, 16),
        "wsk": wshard(Wsk, kvcols, 16),
        "wsv": wshard(Wsv, kvcols, 16),
        "wo": f16(wo_sh.reshape(4, 128, 2048).transpose(1, 0, 2)),
        "mconc": make_mconc(m),
        "ident": np.eye(128, dtype=np.float16),
        "sel": sel,
    }


def gather(results):
    out = np.zeros((2, T, HID), np.float32)
    for core in range(8):
        b = core // 4
        out[b] += results[core]["out_t"].T.astype(np.float32)
    return out


# ----------------------------------------------------------------------------
# Harness entry point
# ----------------------------------------------------------------------------
_NC_CACHE = []


def _get_program():
    if not _NC_CACHE:
        _NC_CACHE.append(build_program())
    return _NC_CACHE[0]


def _run(inp, trace=False, tmpdir=None):
    from concourse.bass_utils import run_bass_kernel_spmd

    nc = _get_program()
    in_maps = [make_inputs(core, **{k: np.asarray(inp[k]) for k in (
        "hidden_states", "ssm_states", "Wq", "Wk", "Wv", "Wsk", "Wsv", "Wo")})
        for core in range(8)]
    res = run_bass_kernel_spmd(nc, in_maps, list(range(8)), trace=trace,
                               tmpdir=tmpdir)
    return gather(res.results), res.exec_time_ns


def kernel(hidden_states, ssm_states, Wq, Wk, Wv, Wsk, Wsv, Wo):
    out, _ = _run(dict(
        hidden_states=hidden_states, ssm_states=ssm_states, Wq=Wq, Wk=Wk,
        Wv=Wv, Wsk=Wsk, Wsv=Wsv, Wo=Wo))
    return out


# revision 49
# speedup vs baseline: 1.0422x; 1.0009x over previous
"""DualSlidingWindowAttention Trainium2 kernel.

Sharding: 8 cores = 2 batches x 4 head-groups. Core (b, m) owns batch b,
q-heads 8m..8m+7, kv-heads 2m, 2m+1. Host sums the 4 partial o-proj outputs
per batch.

Per-core device program (identical SPMD program, per-core data):
  Phase 1: projections with weights stationary -> transposed outputs
           (qT, kT land score-ready; v is DMA-transposed to [kv, D] via the
           HWDGE xbar, keeping the PE free). All xt tiles get distinct SBUF
           buffers so the input stream prefetches the whole run.
  Phase 2: block-sparse attention. Per (kv-group, 128-query tile) only 5
           128-wide kv chunks matter (3 attn-window from hidden + 2
           ssm-window from ssm). Scores are computed transposed [kv, q] with
           the 4 heads of the group interleaved in the free dim (N=512).
           Softmax: exp(s/8) on ACT (no max subtraction; scores bounded),
           then multiplicative mask*exp(alibi) tile on DVE (GPSIMD takes 1
           in 4 units), softmax sums via a ones-column appended to v (free
           on the PE). Normalization is per-qtile-pair: Z rows round-trip
           through a small DRAM tile for the (t,pr,j)->(pr,c,j) relayout,
           reciprocal on DVE, then a K=2 selector matmul broadcasts 1/Z
           across partitions (no per-unit broadcast DMAs).
  Phase 3: o-proj in qtile-pair chunks (N=256) interleaved into the unit
           loop so the PE stays dense (HAM stays unthrottled) and the tail
           after the last attention unit is short. Output is stored f16;
           the host accumulates partials in f32.

All matmul operands are fp16 (1 cycle/row on the PE, FWL weight loads,
half-sized DMA) except the tiny f32 selector broadcast; accumulation is
always fp32 in PSUM; softmax sums and reciprocals stay fp32.
"""

import sys

sys.path.insert(0, "/opt/trn_rl_repo")

import numpy as np
import concourse.bass as bass
import concourse.bacc as bacc
import concourse.mybir as mybir
import concourse.tile as tile

F32 = mybir.dt.float32
F16 = mybir.dt.float16

HID, H, HK, G, D, T = 2048, 32, 8, 4, 64, 1024
W_ATT, W_SSM = 256, 64
NQT = T // 128  # 8 query tiles
KVG = 2         # kv heads (= head groups) per core
HL = 4          # q heads per kv group

# slot order: [attn_left, ssm_left, attn_full, attn_causal, ssm_causal]
SLOT_SRC = [1, 0, 1, 1, 0]       # 1 = hidden (attn window), 0 = ssm
SLOT_CHOFF = [-2, -1, -1, 0, 0]  # kv chunk offset relative to qtile
SLOT_OFF = [-256, -128, -128, 0, 0]
SLOT_WIN = [W_ATT, W_SSM, W_ATT, W_ATT, W_SSM]


def first_slot(qt):
    return {0: 3, 1: 1}.get(qt, 0)


def build_program(debug_dump=False):
    nc = bacc.Bacc("TRN2", target_bir_lowering=False, debug=False)

    xt_ssm = nc.declare_dram_parameter("xt_ssm", [HID, T], F16, isOutput=False)
    xt_hid = nc.declare_dram_parameter("xt_hid", [HID, T], F16, isOutput=False)
    wq = nc.declare_dram_parameter("wq", [4, 128, 32, 128], F16, isOutput=False)
    wk = nc.declare_dram_parameter("wk", [128, 16, 128], F16, isOutput=False)
    wv = nc.declare_dram_parameter("wv", [128, 16, 128], F16, isOutput=False)
    wsk = nc.declare_dram_parameter("wsk", [128, 16, 128], F16, isOutput=False)
    wsv = nc.declare_dram_parameter("wsv", [128, 16, 128], F16, isOutput=False)
    wo = nc.declare_dram_parameter("wo", [128, 4, 2048], F16, isOutput=False)
    mconc = nc.declare_dram_parameter("mconc", [128, 10, 512], F16, isOutput=False)
    ident = nc.declare_dram_parameter("ident", [128, 128], F16, isOutput=False)
    sel = nc.declare_dram_parameter("sel", [2, 128], F32, isOutput=False)
    out_t = nc.declare_dram_parameter("out_t", [HID, T], F16, isOutput=True)
    if debug_dump:
        zrow_d = nc.declare_dram_parameter("zrow_d", [2, 4, T], F32,
                                           isOutput=True)
        oT_d = nc.declare_dram_parameter("oT_d", [128, 4, T], F32,
                                         isOutput=True)
        oTb_d = nc.declare_dram_parameter("oTb_d", [128, 4, T], F16,
                                          isOutput=True)

    mm = nc.tensor.matmul

    with tile.TileContext(nc) as tc:
        with (
            tc.tile_pool(name="persist", bufs=1) as pers,
        ):
            # persistent sbuf tiles
            qT_sb = pers.tile([128, NQT, HL * 128], F16, tag="qT")
            kT_sb = [pers.tile([128, T], F16, tag=f"kT{s}", name=f"kT{s}")
                     for s in range(2)]
            # v_sb[src][kvh]: [tok-in-chunk, chunk, D+1]; col 64 = ones
            v_sb = [
                [pers.tile([128, NQT, 65], F16, tag=f"v{s}{h}", name=f"v{s}{h}")
                 for h in range(2)]
                for s in range(2)
            ]
            sel_sb = pers.tile([2, 128], F32, tag="sel")
            ident_sb = pers.tile([128, 128], F16, tag="ident")
            oT_sb = pers.tile([128, 4, T], F32, tag="oT")
            oTb_sb = pers.tile([128, 4, T], F16, tag="oTb")
            m_sb = pers.tile([128, 10, 512], F16, tag="mconc")
            wo_sb = pers.tile([128, 4, 2048], F16, tag="wo")
            # Z rows relaid to [pr, c=(kvg,t), tok] for the selector broadcast
            zrow_sb = pers.tile([2, 4, T], F32, tag="zrow")

            # ones columns of v (softmax-sum rows) — set once
            for vsrc in range(2):
                for vh in range(2):
                    nc.vector.memset(v_sb[vsrc][vh][:, :, 64:65], 1.0)

            units = [(kvg, qt) for qt in range(NQT) for kvg in range(KVG)]
            wei_tiles = {}

            # SBUF pools spanning attention (overlap region + back half)
            attn_sbuf = (
                tc.tile_pool(name="weip", bufs=3),
                tc.tile_pool(name="ostgp", bufs=2),
                tc.tile_pool(name="outstgp", bufs=3),
                tc.tile_pool(name="recipp", bufs=2),
            )
            weip, ostgp, outstgp, recipp = (p.__enter__() for p in attn_sbuf)

            def emit_scores(u, sp):
                kvg, qt = units[u]
                fs = first_slot(qt)
                wei_t = weip.tile([128, 5, 512], F16, tag="wei")
                wei_tiles[u] = wei_t
                for s in range(fs, 5):
                    ch = qt + SLOT_CHOFF[s]
                    sp_t = sp.tile([128, 512], F32, tag="sp")
                    mm(sp_t[:, :],
                       lhsT=kT_sb[SLOT_SRC[s]][kvg * 64:(kvg + 1) * 64,
                                               ch * 128:(ch + 1) * 128],
                       rhs=qT_sb[kvg * 64:(kvg + 1) * 64, qt, :],
                       start=True, stop=True)
                    nc.scalar.activation(
                        out=wei_t[:, s, :], in_=sp_t[:, :],
                        func=mybir.ActivationFunctionType.Exp, scale=0.125)
                nc.vector.tensor_mul(
                    wei_t[:, fs:5, :], wei_t[:, fs:5, :],
                    m_sb[:, kvg * 5 + fs:kvg * 5 + 5, :])

            def emit_o(u, op, evac=None):
                kvg, qt = units[u]
                fs = first_slot(qt)
                wei_t = wei_tiles.pop(u)
                op_t = op.tile([128, 512], F32, tag="op")
                for s in range(fs, 5):
                    ch = qt + SLOT_CHOFF[s]
                    mm(op_t[0:65, :],
                       lhsT=v_sb[SLOT_SRC[s]][kvg][:, ch, :],
                       rhs=wei_t[:, s, :],
                       start=(s == fs), stop=(s == 4))
                ostg = ostgp.tile([128, 512], F32, tag="ostg")
                if evac is nc.vector:
                    nc.vector.tensor_copy(ostg[0:65, :], op_t[0:65, :])
                else:
                    nc.scalar.copy(ostg[0:65, :], op_t[0:65, :])
                # Z row (free layout (t, pr, j)) -> zrow[pr, (kvg,t), tok].
                # These 4 small DMAs ride the (otherwise idle) SWDGE queue so
                # they don't serialize the sync HWDGE ring (~0.6us apiece).
                zsrc = ostg[64:65, :].rearrange(
                    "p (t pr j) -> p t pr j", t=2, pr=2)
                for par in range(2):
                    nc.gpsimd.dma_start(
                        out=zrow_sb[par:par + 1, kvg * 2:kvg * 2 + 2,
                                    qt * 128:(qt + 1) * 128],
                        in_=zsrc[:, :, par, :])
                for par in range(2):
                    src_ap = ostg[0:64, :].rearrange(
                        "p (t pr j) -> p t pr j", t=2, pr=2)[:, :, par, :]
                    dst_ap = oT_sb[par * 64:(par + 1) * 64,
                                   kvg * 2:kvg * 2 + 2,
                                   qt * 128:(qt + 1) * 128]
                    nc.gpsimd.dma_start(out=dst_ap, in_=src_ap)

            # ---------------- Phase 1 + overlapped attention ----------------
            with (
                tc.tile_pool(name="wqp", bufs=1) as wqp,
                tc.tile_pool(name="xtp", bufs=40) as xtp,
                tc.tile_pool(name="stgp", bufs=2) as stgp,
                tc.tile_pool(name="qp", bufs=4, space="PSUM") as qp,
            ):
                w4_names = ("wsk", "wsv", "wk", "wv")
                w4_t = {"wsk": wsk, "wsv": wsv, "wk": wk, "wv": wv}
                w4_sb = {}
                for name in w4_names:
                    w4_sb[name] = wqp.tile([128, 16, 128], F16, tag=name, name=name)
                wq_sb = [wqp.tile([128, 32, 128], F16, tag=f"wq{c}", name=f"wq{c}")
                         for c in range(4)]

                # DMA emission order = consumption order so the single HWDGE
                # queue streams without head-of-line blocking.
                nc.sync.dma_start(out=w4_sb["wsk"], in_=wsk[:, :, :])
                nc.sync.dma_start(out=w4_sb["wsv"], in_=wsv[:, :, :])
                xts = {}

                def load_xt(half, src):
                    xt_t = xt_hid if src else xt_ssm
                    for kc in range(16):
                        xtile = xtp.tile([128, 512], F16, tag="xt",
                                         name=f"xt{half}_{src}_{kc}")
                        nc.sync.dma_start(
                            out=xtile,
                            in_=xt_t[kc * 128:(kc + 1) * 128,
                                     half * 512:(half + 1) * 512])
                        xts[(half, src, kc)] = xtile

                def load_wq(c):
                    # c-major host layout: each col-tile is one contiguous
                    # 1MB slab (512B+ per descriptor line => full DMA rate)
                    nc.sync.dma_start(
                        out=wq_sb[c][:, :, :],
                        in_=wq[c, :, :, :])

                load_wq(0)
                load_xt(0, 0)
                nc.sync.dma_start(out=w4_sb["wk"], in_=wk[:, :, :])
                nc.sync.dma_start(out=w4_sb["wv"], in_=wv[:, :, :])
                load_wq(1)
                load_xt(0, 1)
                load_wq(2)
                load_wq(3)
                load_xt(1, 0)
                load_xt(1, 1)
                nc.sync.dma_start(out=ident_sb, in_=ident[:, :])
                nc.sync.dma_start(out=m_sb, in_=mconc[:, :, :])
                nc.sync.dma_start(out=sel_sb, in_=sel[:, :])
                for c4 in range(4):
                    nc.sync.dma_start(out=wo_sb[:, c4, :], in_=wo[:, c4, :])

                def emit_q(qps, half, c, src):
                    for kc in range(16):
                        mm(qps[c][:, :],
                           lhsT=wq_sb[c][:, src * 16 + kc, :],
                           rhs=xts[(half, src, kc)][:, :],
                           start=(src == 0 and kc == 0),
                           stop=(src == 1 and kc == 15))

                def evac_q(qps, half):
                    # host permutes Wq cols so col-tile c = [head c (kvg0),
                    # head 4+c (kvg1)] -> partition p maps to p directly.
                    for c in range(4):
                        nc.vector.tensor_copy(
                            qT_sb[:, half * 4:(half + 1) * 4,
                                  c * 128:(c + 1) * 128],
                            qps[c][:, :].rearrange("p (qt j) -> p qt j", j=128))

                qps_h = [
                    [qp.tile([128, 512], F32, tag="qps", name=f"qps{hf}_{i}")
                     for i in range(4)]
                    for hf in range(2)
                ]

                with (
                    tc.tile_pool(name="kvp", bufs=2, space="PSUM") as kvp,
                    tc.tile_pool(name="tp", bufs=2, space="PSUM") as tp,
                ):
                    def emit_kv(half, src, qc0=None):
                        kps = kvp.tile([128, 512], F32, tag="kvps")
                        vps = kvp.tile([128, 512], F32, tag="kvps")
                        wk_t = w4_sb["wk" if src else "wsk"]
                        wv_t = w4_sb["wv" if src else "wsv"]
                        for kc in range(16):
                            xtile = xts[(half, src, kc)]
                            mm(kps[:, :], lhsT=wk_t[:, kc, :], rhs=xtile[:, :],
                               start=(kc == 0), stop=(kc == 15))
                            mm(vps[:, :], lhsT=wv_t[:, kc, :], rhs=xtile[:, :],
                               start=(kc == 0), stop=(kc == 15))
                            if qc0 is not None:
                                # 3rd matmul per xt tile: q col-tile 0 rides
                                # along so the PE is dense from the first
                                # bytes of the input stream.
                                mm(qc0[:, :],
                                   lhsT=wq_sb[0][:, src * 16 + kc, :],
                                   rhs=xtile[:, :],
                                   start=(src == 0 and kc == 0),
                                   stop=(src == 1 and kc == 15))
                        nc.vector.tensor_copy(
                            kT_sb[src][:, half * 512:(half + 1) * 512],
                            kps[:, :])
                        vstg = stgp.tile([128, 512], F16, tag="vstg")
                        nc.vector.tensor_copy(vstg[:, :], vps[:, :])
                        # v -> [tok, D] via PE transpose
                        for h in range(2):
                            for j4 in range(4):
                                tp_t = tp.tile([128, 64], F16, tag="tp")
                                nc.tensor.transpose(
                                    tp_t[:, :],
                                    vstg[h * 64:(h + 1) * 64,
                                         j4 * 128:(j4 + 1) * 128],
                                    ident_sb[h * 64:(h + 1) * 64,
                                             h * 64:(h + 1) * 64])
                                nc.scalar.copy(
                                    v_sb[src][h][:, half * 4 + j4, 0:64],
                                    tp_t[:, :])

                    # half 0 fully (kv + q-c0 fused per tile); half 1 kv only.
                    emit_kv(0, 0, qc0=qps_h[0][0])
                    emit_kv(0, 1, qc0=qps_h[0][0])
                    for c in range(1, 4):
                        emit_q(qps_h[0], 0, c, 0)
                        emit_q(qps_h[0], 0, c, 1)
                    evac_q(qps_h[0], 0)
                    emit_kv(1, 0)
                    emit_kv(1, 1)

                def emit_recip(p):
                    # Z is a positive normal (1 .. ~2e4): approx-fast is safe
                    rc32 = recipp.tile([2, 4, 256], F32, tag="rc32")
                    nc.vector.reciprocal_approx_fast(
                        out=rc32[:, :, :],
                        in_=zrow_sb[0:2, :, p * 256:(p + 1) * 256])
                    return rc32

                # Overlap region: attention units 0..7 (qtiles 0-3, all from
                # half 0) woven with the half-1 q projection, so the PE stays
                # dense through the phase transition (HAM stays warm).
                rc16s = {}
                with (
                    tc.tile_pool(name="spA", bufs=2, space="PSUM") as spA,
                    tc.tile_pool(name="opA", bufs=2, space="PSUM") as opA,
                ):
                    weave = [(0, 0), (0, 1), (1, 0), (1, 1),
                             (2, 0), (2, 1), (3, 0), (3, 1)]
                    for u in range(8):
                        emit_scores(u, spA)
                        c, src = weave[u]
                        emit_q(qps_h[1], 1, c, src)
                        if u > 1:
                            emit_o(u - 2, opA)
                        if u == 7:
                            rc16s[0] = emit_recip(0)
                    emit_o(6, opA)
                    emit_o(7, opA)
                    evac_q(qps_h[1], 1)
                    rc16s[1] = emit_recip(1)

            # ---------------- back half: units 8..15 + all norms/o-proj ----
            with (
                tc.tile_pool(name="spB", bufs=3, space="PSUM") as spB,
                tc.tile_pool(name="opB", bufs=2, space="PSUM") as opB,
                tc.tile_pool(name="rbcp", bufs=1, space="PSUM") as rbcp,
                tc.tile_pool(name="p3", bufs=2, space="PSUM") as p3p,
            ):
                def emit_bcast(p):
                    rc32 = rc16s.pop(p)
                    for c in range(4):
                        rb = rbcp.tile([128, 256], F32, tag="rbc")
                        # selector matmul: out[p, f] = rc[p // 64, c, f]
                        mm(rb[:, :], lhsT=sel_sb[0:2, :], rhs=rc32[0:2, c, :],
                           start=True, stop=True)
                        nc.vector.tensor_mul(
                            oTb_sb[:, c, p * 256:(p + 1) * 256],
                            oT_sb[:, c, p * 256:(p + 1) * 256],
                            rb[:, :])

                def emit_oproj_pair(p, ns=range(16)):
                    for n in ns:
                        ps = p3p.tile([128, 256], F32, tag="p3")
                        for c in range(4):
                            mm(ps[:, :],
                               lhsT=wo_sb[:, c, n * 128:(n + 1) * 128],
                               rhs=oTb_sb[:, c, p * 256:(p + 1) * 256],
                               start=(c == 0), stop=(c == 3))
                        og = outstgp.tile([128, 256], F16, tag="outstg")
                        if n % 2 == 0:
                            nc.scalar.copy(og[:, :], ps[:, :])
                        else:
                            nc.vector.tensor_copy(og[:, :], ps[:, :])
                        nc.sync.dma_start(
                            out=out_t[n * 128:(n + 1) * 128,
                                      p * 256:(p + 1) * 256],
                            in_=og[:, :])

                # oproj tiles double as PE filler: spread across the loop so
                # every norm-chain latency window still has dense PE work
                # (HAM never dips to K=4).
                for u in range(8, 16):
                    emit_scores(u, spB)
                    if u >= 10:
                        emit_o(u - 2, opB, evac=nc.vector)
                    if u == 8:
                        emit_bcast(0)
                        emit_oproj_pair(0, range(0, 8))
                    if u == 9:
                        emit_bcast(1)
                        emit_oproj_pair(0, range(8, 16))
                    if u == 10:
                        emit_oproj_pair(1, range(0, 4))
                    if u == 11:
                        emit_oproj_pair(1, range(4, 8))
                    if u == 12:
                        emit_oproj_pair(1, range(8, 12))
                    if u == 13:
                        rc16s[2] = emit_recip(2)
                        emit_bcast(2)
                        emit_oproj_pair(1, range(12, 16))
                    if u == 14:
                        emit_oproj_pair(2, range(0, 6))
                    if u == 15:
                        emit_oproj_pair(2, range(6, 10))
                emit_o(14, opB, evac=nc.vector)
                emit_oproj_pair(2, range(10, 13))
                emit_o(15, opB, evac=nc.vector)
                emit_oproj_pair(2, range(13, 16))
                rc16s[3] = emit_recip(3)
                emit_bcast(3)
                emit_oproj_pair(3)
                if debug_dump:
                    nc.sync.dma_start(out=zrow_d[:, :, :], in_=zrow_sb[:, :, :])
                    nc.sync.dma_start(out=oT_d[:, :, :], in_=oT_sb[:, :, :])
                    nc.sync.dma_start(out=oTb_d[:, :, :], in_=oTb_sb[:, :, :])

            for p_cm in reversed(attn_sbuf):
                p_cm.__exit__(None, None, None)

    nc.finalize()
    return nc


def make_mconc(m):
    """Mask*exp(alibi) tile for core head-group m: [128, 10, 512] f16."""
    p = np.arange(128)[:, None]
    j = np.arange(128)[None, :]
    out = np.zeros((128, 10, 512), np.float16)
    for kvg in range(KVG):
        for s in range(5):
            rel = SLOT_OFF[s] + p - j  # [128, 128] kv - q
            mask = (-rel >= 0) & (-rel < SLOT_WIN[s])
            for hl in range(HL):
                hg = 8 * m + kvg * 4 + hl
                slope = 2.0 ** (-8.0 * hg / H)
                vals = np.where(mask, np.exp(slope * rel.astype(np.float64)), 0.0)
                out[:, kvg * 5 + s, hl * 128:(hl + 1) * 128] = vals.astype(np.float16)
    return out


def make_inputs(core, hidden_states, ssm_states, Wq, Wk, Wv, Wsk, Wsv, Wo):
    b, m = core // 4, core % 4
    f16 = lambda x: np.ascontiguousarray(np.asarray(x, dtype=np.float16))

    def wshard(W, cols, nchunk):
        # [K, cols] -> [128, K//128, cols]
        Ws = np.asarray(W)[:, cols]
        return f16(Ws.reshape(nchunk, 128, Ws.shape[1]).transpose(1, 0, 2))

    # col-tile c = [head c (kvg0) cols, head 4+c (kvg1) cols]
    qperm = np.concatenate(
        [np.arange(64) + 64 * h for c in range(4) for h in (c, 4 + c)])
    qcols = 512 * m + qperm
    wq_sh = wshard(Wq, qcols, 32)  # [128, 32, 512]
    wq_cmaj = np.ascontiguousarray(
        wq_sh.reshape(128, 32, 4, 128).transpose(2, 0, 1, 3))
    kvcols = slice(128 * m, 128 * (m + 1))
    wo_sh = np.asarray(Wo)[512 * m:512 * (m + 1), :]
    sel = np.zeros((2, 128), np.float32)
    sel[0, 0:64] = 1.0
    sel[1, 64:128] = 1.0
    return {
        "xt_ssm": f16(np.asarray(ssm_states)[b].T),
        "xt_hid": f16(np.asarray(hidden_states)[b].T),
        "wq": wq_cmaj,
        "wk": wshard(Wk, kvcols, 16),
        "wv": wshard(Wv, kvcols, 16),
        "wsk": wshard(Wsk, kvcols, 16),
        "wsv": wshard(Wsv, kvcols, 16),
        "wo": f16(wo_sh.reshape(4, 128, 2048).transpose(1, 0, 2)),
        "mconc": make_mconc(m),
        "ident": np.eye(128, dtype=np.float16),
        "sel": sel,
    }


def gather(results):
    out = np.zeros((2, T, HID), np.float32)
    for core in range(8):
        b = core // 4
        out[b] += results[core]["out_t"].T.astype(np.float32)
    return out


# ----------------------------------------------------------------------------
# Harness entry point
# ----------------------------------------------------------------------------
_NC_CACHE = []


def _get_program():
    if not _NC_CACHE:
        _NC_CACHE.append(build_program())
    return _NC_CACHE[0]


def _run(inp, trace=False, tmpdir=None):
    from concourse.bass_utils import run_bass_kernel_spmd

    nc = _get_program()
    in_maps = [make_inputs(core, **{k: np.asarray(inp[k]) for k in (
        "hidden_states", "ssm_states", "Wq", "Wk", "Wv", "Wsk", "Wsv", "Wo")})
        for core in range(8)]
    res = run_bass_kernel_spmd(nc, in_maps, list(range(8)), trace=trace,
                               tmpdir=tmpdir)
    return gather(res.results), res.exec_time_ns


def kernel(hidden_states, ssm_states, Wq, Wk, Wv, Wsk, Wsv, Wo):
    out, _ = _run(dict(
        hidden_states=hidden_states, ssm_states=ssm_states, Wq=Wq, Wk=Wk,
        Wv=Wv, Wsk=Wsk, Wsv=Wsv, Wo=Wo))
    return out
